# revision 52
# baseline (speedup 1.0000x reference)
"""Trainium2 Bass kernel for nn_FEDformerEncoder (8-core data parallel).

The reference network is, per layer (L=2):
    y  = mean_e( conv1d_same(x, w_e) + b_e )              (depthwise conv on W)
    q,k,v = y @ w{q,k,v}.T + b{q,k,v}                     ([rows, P])
    Q,K,V = fft(q),fft(k),fft(v)
    Wt = K * conj(Q) / sqrt(P) * V
    out = ifft(Wt).real @ wo.T + bo

Everything except the elementwise complex triple product is linear in x, so
the conv, the FFT, and the iFFT fold into host-precomputed projection
weights.  Real-input FFT symmetry packs each 1024-bin complex spectrum into
exactly 1024 reals per signal: block A = Re[0..511], block B =
[Re[512](Nyquist), Im[1..511]].  Composing the two layers' linear maps
(iFFT-projection of layer 1 directly into conv+FFT-projection of layer 2)
collapses the whole network into three matmul stages and two elementwise
stages:

    S1  = x   @ EW1  + b1     # [rows,2048] @ [2048,3072]
    Wt1 = complex-triple(S1)  # packed; slot 0 of A/B = DC/Nyquist, real
    S2  = Wt1 @ M12  + b2     # [rows,1024] @ [1024,3072], M12 = WoP1@EW2
    Wt2 = complex-triple(S2)
    out = Wt2 @ WoP2 + bo2    # [rows,1024] @ [1024,2048]

Sharded batch-wise over 8 cores (4 batches = 512 rows per core), weights
replicated.  Activations live in SBUF in transposed layout [feature(part),
row(free)] throughout, so no on-device transposes are needed.  Matmul
operands are fp16 (fp32 PSUM accumulation, fp32 elementwise); contraction
row-tiles are interleaved [A0 B0 A1 B1 ...] so each elementwise group
feeds the next stage in production order and the stages pipeline.

The kernel is PE-bound: 704 matmul tiles x 512 fp16 rows = 150.2us of
tensor-engine work at 2.4GHz, so everything else is scheduled around a
gapless PE run:

- HAM warm-up: dummy matmuls (no DMA deps) start right after the ~7.2us
  runtime prologue so the PE's duty-cycle clock gate (cold K=4/8)
  releases around the time the first real data lands.
- Group t=0 of stage 1 runs all six output tiles as ONE 6-way
  interleaved k-loop: per 128KB x k-slice the PE does 6 matmuls, so x
  HBM demand is ~100GB/s and weights ~150GB/s — comfortably under the
  ~358GB/s HBM-per-core limit while x streams in.
- Weights are pre-grouped in DRAM by elementwise group (host repack)
  and stream in k-need order: small first chunks (PE start ~11.5us),
  then large transfers.  Later groups' loads are time-gated
  (tile_wait_until) so the Tile scheduler cannot hoist them ahead of
  the critical stream and oversubscribe HBM — sustained >300GB/s DMA
  also risks the chip's P0 power downclock (PE 2.4 -> 2.0GHz).
- Output is written fp16 (cast to fp32 on host); rel err stays ~1.2e-3.
"""
import sys

import numpy as np

sys.path.insert(0, "/opt/trn_rl_repo")

import concourse.bass as bass
import concourse.mybir as mybir
import concourse.tile as tile
from concourse import bacc
from concourse.bass_utils import run_bass_kernel_spmd

BS, CNT, W, P, E, KK, L = 32, 128, 2048, 1024, 8, 25, 2
H = P // 2                    # 512 slots per packed block
NCORES = 8
ROWS = (BS // NCORES) * CNT   # 512 rows per core
KT = W // 128                 # 16 contraction tiles (stage 1)
MT = (3 * P) // 128           # 24 output tiles (stages 1,2: q|k|v packed)
ST = P // 128                 # 8 contraction tiles (stages 2,3)
WT = W // 128                 # 16 output tiles (stage 3)
F32 = mybir.dt.float32
ACT = mybir.dt.float16
ACT_NP = np.float16
IDENT = mybir.ActivationFunctionType.Identity


def _fold_layer(conv_w, conv_b, wq, bq, wk, bk, wv, bv, wo, bo):
    """Fold conv + FFT into projection weights (float64 math).

    Returns EW [W, 3*P], Sbias [3*P], WoP [P, W] (rows interleaved
    [A0 B0 A1 B1 A2 B2 A3 B3] by 128-tile), bo [W].
    """
    f64 = np.float64
    wbar = conv_w.astype(f64).mean(axis=0)[0]          # [KK]
    bbar = conv_b.astype(f64).mean()

    idx = np.arange(W)
    D = idx[None, :] - idx[:, None] + (KK // 2)        # C[w,u] = wbar[u-w+12]
    C = np.where((D >= 0) & (D < KK), wbar[np.clip(D, 0, KK - 1)], 0.0)

    def pack_fwd(wmat, bvec, scale=1.0):
        Wf = np.fft.fft(wmat.astype(f64), axis=0)      # [P, W]
        Bf = np.fft.fft(bvec.astype(f64))              # [P]
        cols = np.empty((W, P), dtype=f64)
        cols[:, :H] = Wf[:H, :].real.T
        cols[:, H] = Wf[H, :].real
        cols[:, H + 1:] = Wf[1:H, :].imag.T
        bias = np.empty(P, dtype=f64)
        bias[:H] = Bf[:H].real
        bias[H] = Bf[H].real
        bias[H + 1:] = Bf[1:H].imag
        return cols * scale, bias * scale

    s = 1.0 / np.sqrt(f64(P))
    cq, bq_p = pack_fwd(wq, bq)
    ck, bk_p = pack_fwd(wk, bk)
    cv, bv_p = pack_fwd(wv, bv, scale=s)
    cols = np.concatenate([cq, ck, cv], axis=1)        # [W, 3P]
    bias = np.concatenate([bq_p, bk_p, bv_p])

    EW = C.T @ cols
    Sbias = bbar * cols.sum(axis=0) + bias

    G = np.fft.ifft(wo.astype(f64), axis=1)            # [W, P]
    WoP = np.empty((P, W), dtype=f64)
    WoP[0] = G[:, 0].real
    WoP[1:H] = 2.0 * G[:, 1:H].real.T
    WoP[H] = G[:, H].real
    WoP[H + 1:] = -2.0 * G[:, 1:H].imag.T
    # interleave row-tiles A0 B0 A1 B1 ... to match Wcat production order
    WoP = WoP.reshape(2, 4, 128, W).transpose(1, 0, 2, 3).reshape(P, W)

    return EW, Sbias, WoP, bo.astype(f64)


def _build_module():
    nc = bacc.Bacc("TRN2", target_bir_lowering=False, debug=False)

    # x, k-pair-major: col = kp*1024 + half*512 + row  (k = 2*kp + half)
    xin = nc.dram_tensor("xin", [128, KT * ROWS], ACT, kind="ExternalInput")
    # stage-1 weights grouped by elementwise group t: col = b*2048 + k*128
    ew1g = nc.dram_tensor("ew1g", [4, 128, 6 * W], ACT, kind="ExternalInput")
    # stage-2 weights grouped by elementwise group t2: col = b*1024 + s*128
    m12g = nc.dram_tensor("m12g", [4, 128, 6 * P], ACT, kind="ExternalInput")
    # stage-3 weights in 2 chunks of 8 out-tiles: col = jj*1024 + s*128
    wo2g = nc.dram_tensor("wo2g", [2, 128, 8 * P], ACT, kind="ExternalInput")
    # [sb1(24) | sb2(24) | bo2(16)] packed into one [128, 64] tensor
    biases = nc.dram_tensor("biases", [128, 2 * MT + WT], F32,
                            kind="ExternalInput")
    # output pair-major: xout[jp][:, i*ROWS + r] = out tile j=2*jp+i
    xout = nc.dram_tensor("xout", [WT // 2, 128, 2 * ROWS], ACT,
                          kind="ExternalOutput")

    with tile.TileContext(nc) as tc:
        with (
            tc.tile_pool(name="xbuf", bufs=1) as xpool,
            tc.tile_pool(name="bias", bufs=1) as bpool,
            tc.tile_pool(name="wq0", bufs=1) as wqpool,
            tc.tile_pool(name="wg", bufs=2) as wgpool,
            tc.tile_pool(name="wm", bufs=2) as wmpool,
            tc.tile_pool(name="wo2", bufs=2) as wo2pool,
            tc.tile_pool(name="spec", bufs=10) as spool,
            tc.tile_pool(name="wt", bufs=16) as wtpool,
            tc.tile_pool(name="ew", bufs=6) as ewpool,
            tc.tile_pool(name="out", bufs=4) as opool,
            tc.tile_pool(name="psum", bufs=8, space="PSUM") as pspool,
        ):
            # first stage-1 weight blocks go out before x so the PE can
            # start as soon as x k-tiles stream in; x loads issue on the
            # (idle-at-start) scalar engine's DGE, weights on sync's.
            # The two t=0 tiles' chunks are issue-interleaved so BOTH have
            # their first half-tile early: the t=0 k-loop alternates j=0/j=4
            # matmuls, so it stalls on whichever tile lands later.  Each
            # DMA instruction costs ~0.65us of issue time on its engine, so
            # chunks are as few and as large as latency allows.
            # PE warm-up: the HAM clock gate holds a cold tensor engine at
            # half duty (K=4/8) until it sees several us of sustained
            # activity.  These matmuls depend on no DMA, so they run right
            # after the runtime prologue and release the gate around the
            # time real data lands.
            scratch = bpool.tile([128, ROWS], ACT, tag="warm")
            nc.gpsimd.memset(scratch[:], 0.0)
            wps = pspool.tile([128, ROWS], F32, tag="ps")
            for _ in range(13):
                nc.tensor.matmul(wps[:], scratch[:, 0:128], scratch[:],
                                 start=True, stop=True)

            # t=0 runs ALL SIX tiles as one 6-way-interleaved k-loop: per k
            # step that's 6 matmuls per 128KB x slice, so x HBM demand is
            # ~100GB/s and weights ~150GB/s — comfortably under the
            # ~358GB/s HBM-per-core limit even at full PE clock, leaving
            # slack for DMA jitter.  Chunks are small and k-need-ordered
            # (all six tiles' first 512 cols, then k-range halves) so the
            # k-loop starts early and never outruns the stream.  b=0..3 on
            # sync, b=4..5 + x on scalar.
            wq0 = wqpool.tile([128, 6 * W], ACT, tag="wq")
            xall = xpool.tile([128, KT * ROWS], ACT, tag="x")
            qx = ROWS * 2

            def wchunk(eng, b, c0, c1):
                eng.dma_start(wq0[:, b * W + c0 * 128:][:, :(c1 - c0) * 128],
                              ew1g[0][:, b * W + c0 * 128:][:, :(c1 - c0) * 128])

            def xchunk(c0, c1):
                nc.scalar.dma_start(xall[:, c0 * qx:c1 * qx],
                                    xin[:, c0 * qx:c1 * qx])

            for b in range(4):
                wchunk(nc.sync, b, 0, 4)         # k0-3 cols, 128KB each
            xchunk(0, 1)                          # k0,k1
            wchunk(nc.scalar, 4, 0, 4)
            wchunk(nc.scalar, 5, 0, 4)
            xchunk(1, 3)                          # k2-5
            for b in range(4):
                wchunk(nc.sync, b, 4, 10)        # k4-9 cols, 192KB each
            wchunk(nc.scalar, 4, 4, 10)
            wchunk(nc.scalar, 5, 4, 10)
            xchunk(3, 6)                          # k6-11
            for b in range(4):
                wchunk(nc.sync, b, 10, 16)       # k10-15 cols
            wchunk(nc.scalar, 4, 10, 16)
            wchunk(nc.scalar, 5, 10, 16)
            xchunk(6, 8)                          # k12-15

            # t=1..3 weight groups, time-gated so the scheduler streams
            # them just ahead of consumption; t=1's chunks are finer so its
            # first tiles complete before group t=0 finishes
            wgt = {}
            for t in (1, 2, 3):
                wg = wgpool.tile([128, 6 * W], ACT, tag="wg")
                if t == 1:
                    with tc.tile_wait_until(0.010):
                        for c in range(4):
                            nc.sync.dma_start(
                                wg[:, bass.ts(c, 3 * W // 2)],
                                ew1g[t][:, bass.ts(c, 3 * W // 2)])
                else:
                    with tc.tile_wait_until(0.012 + 0.020 * (t - 1)):
                        nc.sync.dma_start(wg[:, :3 * W], ew1g[t][:, :3 * W])
                        nc.sync.dma_start(wg[:, 3 * W:], ew1g[t][:, 3 * W:])
                wgt[t] = wg

            def xslice(k):
                return xall[:, (k // 2) * 1024 + (k % 2) * ROWS:][:, :ROWS]

            btile = bpool.tile([128, 2 * MT + WT], F32, tag="biases")
            nc.gpsimd.dma_start(btile[:], biases[:])

            def sb1_col(j):
                return btile[:, j:j + 1]

            def sb2_col(j):
                return btile[:, MT + j:MT + j + 1]

            def bo2_col(j):
                return btile[:, 2 * MT + j:2 * MT + j + 1]

            def elementwise(St, first):
                """complex triple product on one partition-row group."""
                qA, qB, kA, kB, vA, vB = St
                v = nc.vector
                cr = ewpool.tile([128, ROWS], F32, tag="ew")
                ci = ewpool.tile([128, ROWS], F32, tag="ew")
                t0 = ewpool.tile([128, ROWS], F32, tag="ew")
                v.tensor_mul(cr[:], kA[:], qA[:])
                v.tensor_mul(t0[:], kB[:], qB[:])
                v.tensor_add(cr[:], cr[:], t0[:])
                v.tensor_mul(ci[:], kB[:], qA[:])
                v.tensor_mul(t0[:], kA[:], qB[:])
                v.tensor_sub(ci[:], ci[:], t0[:])
                wr = wtpool.tile([128, ROWS], ACT, tag="wt")
                wi = wtpool.tile([128, ROWS], ACT, tag="wt")
                v.tensor_mul(wr[:], cr[:], vA[:])
                v.tensor_mul(t0[:], ci[:], vB[:])
                v.tensor_sub(wr[:], wr[:], t0[:])
                v.tensor_mul(wi[:], cr[:], vB[:])
                v.tensor_mul(t0[:], ci[:], vA[:])
                v.tensor_add(wi[:], wi[:], t0[:])
                if first:
                    # slot 0: A holds DC, B holds Nyquist — both real
                    v.tensor_mul(t0[0:1, :], qA[0:1, :], kA[0:1, :])
                    v.tensor_mul(wr[0:1, :], t0[0:1, :], vA[0:1, :])
                    v.tensor_mul(t0[0:1, :], qB[0:1, :], kB[0:1, :])
                    v.tensor_mul(wi[0:1, :], t0[0:1, :], vB[0:1, :])
                return wr, wi

            # ---- stage 1: S1 = x @ EW1 + b1, pipelined elementwise ----
            Wcat1 = [None] * ST
            for t in range(4):
                St = []
                if t == 0:
                    # 6-way-interleaved k-loop while x streams
                    psq = []
                    for _ in range(6):
                        ps = pspool.tile([128, ROWS], F32, tag="ps")
                        psq.append(ps)
                    for k in range(KT):
                        for i in range(6):
                            nc.tensor.matmul(
                                psq[i][:], wq0[:, i * W + k * 128:][:, :128],
                                xslice(k), start=(k == 0), stop=(k == KT - 1))
                    for i in range(6):
                        Sj = spool.tile([128, ROWS], F32, tag="spec")
                        nc.scalar.activation(Sj[:], psq[i][:], IDENT,
                                             bias=sb1_col(4 * i))
                        St.append(Sj)
                else:
                    for b in range(6):   # qA qB kA kB vA vB row t
                        j = b * 4 + t
                        ps = pspool.tile([128, ROWS], F32, tag="ps")
                        for k in range(KT):
                            nc.tensor.matmul(
                                ps[:],
                                wgt[t][:, b * W + k * 128:][:, :128],
                                xslice(k), start=(k == 0), stop=(k == KT - 1))
                        Sj = spool.tile([128, ROWS], F32, tag="spec")
                        nc.scalar.activation(Sj[:], ps[:], IDENT,
                                             bias=sb1_col(j))
                        St.append(Sj)
                wr, wi = elementwise(St, t == 0)
                Wcat1[2 * t] = wr
                Wcat1[2 * t + 1] = wi

            # stage-2/3 weights prefetch as few large transfers, time-gated
            # (tile_wait_until holds them back in the Tile scheduler) so
            # they can't jump ahead of the stage-1 weight stream and steal
            # HBM bandwidth from the critical path
            w2t = []
            for t in range(4):
                w2 = wmpool.tile([128, 6 * P], ACT, tag="wm")
                with tc.tile_wait_until(0.048 + 0.008 * t):
                    nc.sync.dma_start(w2[:], m12g[t])
                w2t.append(w2)
            wo2t = []
            for c in range(2):
                wo2 = wo2pool.tile([128, 8 * P], ACT, tag="wo2")
                with tc.tile_wait_until(0.085 + 0.010 * c):
                    nc.scalar.dma_start(wo2[:], wo2g[c])
                wo2t.append(wo2)

            # ---- stage 2: S2 = Wt1 @ M12 + b2, pipelined elementwise ----
            Wcat2 = [None] * ST
            for t in range(4):
                St = []
                for b in range(6):
                    j = b * 4 + t
                    ps = pspool.tile([128, ROWS], F32, tag="ps")
                    for s in range(ST):
                        nc.tensor.matmul(
                            ps[:], w2t[t][:, b * P + s * 128:][:, :128],
                            Wcat1[s][:], start=(s == 0), stop=(s == ST - 1))
                    Sj = spool.tile([128, ROWS], F32, tag="spec")
                    nc.scalar.activation(Sj[:], ps[:], IDENT,
                                         bias=sb2_col(j))
                    St.append(Sj)
                wr, wi = elementwise(St, t == 0)
                Wcat2[2 * t] = wr
                Wcat2[2 * t + 1] = wi

            # ---- stage 3: out = Wt2 @ WoP2 + bo2 ----
            # output tiles drain in PAIRS into one SBUF tile (one DMA per
            # pair halves the out-DMA count and the teardown walk);
            # alternate drain engines so consecutive tiles' bias-add +
            # copy pipeline instead of serializing on one engine
            for jp in range(WT // 2):
                last_pair = jp == WT // 2 - 1
                ostage = opool.tile([128, 2 * ROWS], ACT, tag="out")
                for i in range(2):
                    j = 2 * jp + i
                    ps = pspool.tile([128, ROWS], F32, tag="ps")
                    for s in range(ST):
                        nc.tensor.matmul(
                            ps[:],
                            wo2t[j // 8][:, (j % 8) * P + s * 128:][:, :128],
                            Wcat2[s][:], start=(s == 0), stop=(s == ST - 1))
                    half = ostage[:, i * ROWS:][:, :ROWS]
                    if i:
                        nc.vector.tensor_scalar_add(half, ps[:], bo2_col(j))
                    else:
                        nc.scalar.activation(half, ps[:], IDENT,
                                             bias=bo2_col(j))
                    if last_pair:
                        # final tiles ship as two half-DMAs on BOTH queues
                        # so the drain chains overlap and the last transfer
                        # is small — this chain is the exec-time endpoint
                        (nc.sync if i == 0 else nc.scalar).dma_start(
                            xout[jp][:, i * ROWS:][:, :ROWS], half)
                if not last_pair:
                    (nc.scalar if jp % 2 else nc.sync).dma_start(
                        xout[jp], ostage[:])
    nc.compile()
    return nc


_MODULE_CACHE = {}


def _get_module():
    if "nc" not in _MODULE_CACHE:
        _MODULE_CACHE["nc"] = _build_module()
    return _MODULE_CACHE["nc"]


def _prepare_weight_maps(conv_w, conv_b, wq, bq, wk, bk, wv, bv, wo, bo):
    folds = [_fold_layer(conv_w[l], conv_b[l], wq[l], bq[l], wk[l], bk[l],
                         wv[l], bv[l], wo[l], bo[l]) for l in range(L)]
    EW1, Sb1, WoP1, _bo1 = folds[0]
    EW2, Sb2, WoP2, bo2 = folds[1]
    M12 = WoP1 @ EW2                               # [P, 3P], fp64
    Sb2e = _bo1 @ EW2 + Sb2                        # [3P]

    # ew1 grouped by elementwise group t (j = b*4 + t, b-major):
    # ew1g[t][p, b*2048 + k*128 + c] = EW1[k*128+p, (b*4+t)*128+c]
    arr1 = EW1.reshape(KT, 128, MT // 4, 4, 128)
    ew1g = np.ascontiguousarray(
        arr1.transpose(3, 1, 2, 0, 4).reshape(4, 128, 6 * W).astype(ACT_NP))

    # m12 grouped by elementwise group t (j = b*4 + t, b-major):
    # m12g[t][p, b*1024 + s*128 + c] = M12[s*128+p, (b*4+t)*128+c]
    arr2 = M12.reshape(ST, 128, MT // 4, 4, 128)
    m12g = np.ascontiguousarray(
        arr2.transpose(3, 1, 2, 0, 4).reshape(4, 128, 6 * P).astype(ACT_NP))

    # wop2 in 2 chunks of 8 out-tiles:
    # wo2g[c][p, jj*1024 + s*128 + cc] = WoP2[s*128+p, (8c+jj)*128+cc]
    arr3 = WoP2.reshape(ST, 128, 2, 8, 128)
    wo2g = np.ascontiguousarray(
        arr3.transpose(2, 1, 3, 0, 4).reshape(2, 128, 8 * P).astype(ACT_NP))

    biases = np.concatenate([
        Sb1.reshape(MT, 128).T, Sb2e.reshape(MT, 128).T,
        bo2.reshape(WT, 128).T], axis=1).astype(np.float32)
    return {
        "ew1g": ew1g,
        "m12g": m12g,
        "wo2g": wo2g,
        "biases": np.ascontiguousarray(biases),
    }


def _make_in_maps(inputs):
    x = np.asarray(inputs["x"], dtype=np.float32)
    wmap = _prepare_weight_maps(
        np.asarray(inputs["conv_w"]), np.asarray(inputs["conv_b"]),
        np.asarray(inputs["wq"]), np.asarray(inputs["bq"]),
        np.asarray(inputs["wk"]), np.asarray(inputs["bk"]),
        np.asarray(inputs["wv"]), np.asarray(inputs["bv"]),
        np.asarray(inputs["wo"]), np.asarray(inputs["bo"]))
    per_core = BS // NCORES
    in_maps = []
    for c in range(NCORES):
        xc = x[c * per_core:(c + 1) * per_core].reshape(ROWS, W)
        xin = np.ascontiguousarray(
            xc.reshape(ROWS, KT // 2, 2, 128).transpose(3, 1, 2, 0)
            .reshape(128, KT * ROWS).astype(ACT_NP))
        in_maps.append({"xin": xin, **wmap})
    return in_maps


def kernel(x, conv_w, conv_b, wq, bq, wk, bk, wv, bv, wo, bo):
    in_maps = _make_in_maps(dict(
        x=x, conv_w=conv_w, conv_b=conv_b, wq=wq, bq=bq, wk=wk, bk=bk,
        wv=wv, bv=bv, wo=wo, bo=bo))
    nc = _get_module()
    res = run_bass_kernel_spmd(nc, in_maps, list(range(NCORES)))

    per_core = BS // NCORES
    outs = []
    for c in range(NCORES):
        xo = res.results[c]["xout"]                    # [WT//2, 128, 2*ROWS]
        xo = (xo.reshape(WT // 2, 128, 2, ROWS).transpose(0, 2, 1, 3)
              .reshape(WT, 128, ROWS))
        outs.append(xo.transpose(2, 0, 1).reshape(per_core, CNT, W))
    return np.concatenate(outs, axis=0).astype(np.float32)



# revision 53
# speedup vs baseline: 1.0001x; 1.0001x over previous
"""Trainium2 Bass kernel for nn_FEDformerEncoder (8-core data parallel).

The reference network is, per layer (L=2):
    y  = mean_e( conv1d_same(x, w_e) + b_e )              (depthwise conv on W)
    q,k,v = y @ w{q,k,v}.T + b{q,k,v}                     ([rows, P])
    Q,K,V = fft(q),fft(k),fft(v)
    Wt = K * conj(Q) / sqrt(P) * V
    out = ifft(Wt).real @ wo.T + bo

Everything except the elementwise complex triple product is linear in x, so
the conv, the FFT, and the iFFT fold into host-precomputed projection
weights.  Real-input FFT symmetry packs each 1024-bin complex spectrum into
exactly 1024 reals per signal: block A = Re[0..511], block B =
[Re[512](Nyquist), Im[1..511]].  Composing the two layers' linear maps
(iFFT-projection of layer 1 directly into conv+FFT-projection of layer 2)
collapses the whole network into three matmul stages and two elementwise
stages:

    S1  = x   @ EW1  + b1     # [rows,2048] @ [2048,3072]
    Wt1 = complex-triple(S1)  # packed; slot 0 of A/B = DC/Nyquist, real
    S2  = Wt1 @ M12  + b2     # [rows,1024] @ [1024,3072], M12 = WoP1@EW2
    Wt2 = complex-triple(S2)
    out = Wt2 @ WoP2 + bo2    # [rows,1024] @ [1024,2048]

Sharded batch-wise over 8 cores (4 batches = 512 rows per core), weights
replicated.  Activations live in SBUF in transposed layout [feature(part),
row(free)] throughout, so no on-device transposes are needed.  Matmul
operands are fp16 (fp32 PSUM accumulation, fp32 elementwise); contraction
row-tiles are interleaved [A0 B0 A1 B1 ...] so each elementwise group
feeds the next stage in production order and the stages pipeline.

The kernel is PE-bound: 704 matmul tiles x 512 fp16 rows = 150.2us of
tensor-engine work at 2.4GHz, so everything else is scheduled around a
gapless PE run:

- HAM warm-up: dummy matmuls (no DMA deps) start right after the ~7.2us
  runtime prologue so the PE's duty-cycle clock gate (cold K=4/8)
  releases around the time the first real data lands.
- Group t=0 of stage 1 runs all six output tiles as ONE 6-way
  interleaved k-loop: per 128KB x k-slice the PE does 6 matmuls, so x
  HBM demand is ~100GB/s and weights ~150GB/s — comfortably under the
  ~358GB/s HBM-per-core limit while x streams in.
- Weights are pre-grouped in DRAM by elementwise group (host repack)
  and stream in k-need order: small first chunks (PE start ~11.5us),
  then large transfers.  Later groups' loads are time-gated
  (tile_wait_until) so the Tile scheduler cannot hoist them ahead of
  the critical stream and oversubscribe HBM — sustained >300GB/s DMA
  also risks the chip's P0 power downclock (PE 2.4 -> 2.0GHz).
- Output is written fp16 (cast to fp32 on host); rel err stays ~1.2e-3.
"""
import sys

import numpy as np

sys.path.insert(0, "/opt/trn_rl_repo")

import concourse.bass as bass
import concourse.mybir as mybir
import concourse.tile as tile
from concourse import bacc
from concourse.bass_utils import run_bass_kernel_spmd

BS, CNT, W, P, E, KK, L = 32, 128, 2048, 1024, 8, 25, 2
H = P // 2                    # 512 slots per packed block
NCORES = 8
ROWS = (BS // NCORES) * CNT   # 512 rows per core
KT = W // 128                 # 16 contraction tiles (stage 1)
MT = (3 * P) // 128           # 24 output tiles (stages 1,2: q|k|v packed)
ST = P // 128                 # 8 contraction tiles (stages 2,3)
WT = W // 128                 # 16 output tiles (stage 3)
F32 = mybir.dt.float32
ACT = mybir.dt.float16
ACT_NP = np.float16
IDENT = mybir.ActivationFunctionType.Identity


def _fold_layer(conv_w, conv_b, wq, bq, wk, bk, wv, bv, wo, bo):
    """Fold conv + FFT into projection weights (float64 math).

    Returns EW [W, 3*P], Sbias [3*P], WoP [P, W] (rows interleaved
    [A0 B0 A1 B1 A2 B2 A3 B3] by 128-tile), bo [W].
    """
    f64 = np.float64
    wbar = conv_w.astype(f64).mean(axis=0)[0]          # [KK]
    bbar = conv_b.astype(f64).mean()

    idx = np.arange(W)
    D = idx[None, :] - idx[:, None] + (KK // 2)        # C[w,u] = wbar[u-w+12]
    C = np.where((D >= 0) & (D < KK), wbar[np.clip(D, 0, KK - 1)], 0.0)

    def pack_fwd(wmat, bvec, scale=1.0):
        Wf = np.fft.fft(wmat.astype(f64), axis=0)      # [P, W]
        Bf = np.fft.fft(bvec.astype(f64))              # [P]
        cols = np.empty((W, P), dtype=f64)
        cols[:, :H] = Wf[:H, :].real.T
        cols[:, H] = Wf[H, :].real
        cols[:, H + 1:] = Wf[1:H, :].imag.T
        bias = np.empty(P, dtype=f64)
        bias[:H] = Bf[:H].real
        bias[H] = Bf[H].real
        bias[H + 1:] = Bf[1:H].imag
        return cols * scale, bias * scale

    s = 1.0 / np.sqrt(f64(P))
    cq, bq_p = pack_fwd(wq, bq)
    ck, bk_p = pack_fwd(wk, bk)
    cv, bv_p = pack_fwd(wv, bv, scale=s)
    cols = np.concatenate([cq, ck, cv], axis=1)        # [W, 3P]
    bias = np.concatenate([bq_p, bk_p, bv_p])

    EW = C.T @ cols
    Sbias = bbar * cols.sum(axis=0) + bias

    G = np.fft.ifft(wo.astype(f64), axis=1)            # [W, P]
    WoP = np.empty((P, W), dtype=f64)
    WoP[0] = G[:, 0].real
    WoP[1:H] = 2.0 * G[:, 1:H].real.T
    WoP[H] = G[:, H].real
    WoP[H + 1:] = -2.0 * G[:, 1:H].imag.T
    # interleave row-tiles A0 B0 A1 B1 ... to match Wcat production order
    WoP = WoP.reshape(2, 4, 128, W).transpose(1, 0, 2, 3).reshape(P, W)

    return EW, Sbias, WoP, bo.astype(f64)


def _build_module():
    nc = bacc.Bacc("TRN2", target_bir_lowering=False, debug=False)

    # x, k-pair-major: col = kp*1024 + half*512 + row  (k = 2*kp + half)
    xin = nc.dram_tensor("xin", [128, KT * ROWS], ACT, kind="ExternalInput")
    # stage-1 weights grouped by elementwise group t: col = b*2048 + k*128
    ew1g = nc.dram_tensor("ew1g", [4, 128, 6 * W], ACT, kind="ExternalInput")
    # stage-2 weights grouped by elementwise group t2: col = b*1024 + s*128
    m12g = nc.dram_tensor("m12g", [4, 128, 6 * P], ACT, kind="ExternalInput")
    # stage-3 weights in 2 chunks of 8 out-tiles: col = jj*1024 + s*128
    wo2g = nc.dram_tensor("wo2g", [2, 128, 8 * P], ACT, kind="ExternalInput")
    # [sb1(24) | sb2(24) | bo2(16)] packed into one [128, 64] tensor
    biases = nc.dram_tensor("biases", [128, 2 * MT + WT], F32,
                            kind="ExternalInput")
    # output pair-major: xout[jp][:, i*ROWS + r] = out tile j=2*jp+i
    xout = nc.dram_tensor("xout", [WT // 2, 128, 2 * ROWS], ACT,
                          kind="ExternalOutput")

    with tile.TileContext(nc) as tc:
        with (
            tc.tile_pool(name="xbuf", bufs=1) as xpool,
            tc.tile_pool(name="bias", bufs=1) as bpool,
            tc.tile_pool(name="wq0", bufs=1) as wqpool,
            tc.tile_pool(name="wg", bufs=2) as wgpool,
            tc.tile_pool(name="wm", bufs=2) as wmpool,
            tc.tile_pool(name="wo2", bufs=2) as wo2pool,
            tc.tile_pool(name="spec", bufs=10) as spool,
            tc.tile_pool(name="wt", bufs=16) as wtpool,
            tc.tile_pool(name="ew", bufs=6) as ewpool,
            tc.tile_pool(name="out", bufs=4) as opool,
            tc.tile_pool(name="psum", bufs=8, space="PSUM") as pspool,
        ):
            # first stage-1 weight blocks go out before x so the PE can
            # start as soon as x k-tiles stream in; x loads issue on the
            # (idle-at-start) scalar engine's DGE, weights on sync's.
            # The two t=0 tiles' chunks are issue-interleaved so BOTH have
            # their first half-tile early: the t=0 k-loop alternates j=0/j=4
            # matmuls, so it stalls on whichever tile lands later.  Each
            # DMA instruction costs ~0.65us of issue time on its engine, so
            # chunks are as few and as large as latency allows.
            # PE warm-up: the HAM clock gate holds a cold tensor engine at
            # half duty (K=4/8) until it sees several us of sustained
            # activity.  These matmuls depend on no DMA, so they run right
            # after the runtime prologue and release the gate around the
            # time real data lands.
            scratch = bpool.tile([128, ROWS], ACT, tag="warm")
            nc.gpsimd.memset(scratch[:], 0.0)
            wps = pspool.tile([128, ROWS], F32, tag="ps")
            for _ in range(13):
                nc.tensor.matmul(wps[:], scratch[:, 0:128], scratch[:],
                                 start=True, stop=True)

            # t=0 runs ALL SIX tiles as one 6-way-interleaved k-loop: per k
            # step that's 6 matmuls per 128KB x slice, so x HBM demand is
            # ~100GB/s and weights ~150GB/s — comfortably under the
            # ~358GB/s HBM-per-core limit even at full PE clock, leaving
            # slack for DMA jitter.  Chunks are small and k-need-ordered
            # (all six tiles' first 512 cols, then k-range halves) so the
            # k-loop starts early and never outruns the stream.  b=0..3 on
            # sync, b=4..5 + x on scalar.
            wq0 = wqpool.tile([128, 6 * W], ACT, tag="wq")
            xall = xpool.tile([128, KT * ROWS], ACT, tag="x")
            qx = ROWS * 2

            def wchunk(eng, b, c0, c1):
                eng.dma_start(wq0[:, b * W + c0 * 128:][:, :(c1 - c0) * 128],
                              ew1g[0][:, b * W + c0 * 128:][:, :(c1 - c0) * 128])

            def xchunk(c0, c1):
                nc.scalar.dma_start(xall[:, c0 * qx:c1 * qx],
                                    xin[:, c0 * qx:c1 * qx])

            for b in range(4):
                wchunk(nc.sync, b, 0, 4)         # k0-3 cols, 128KB each
            xchunk(0, 1)                          # k0,k1
            wchunk(nc.scalar, 4, 0, 4)
            wchunk(nc.scalar, 5, 0, 4)
            xchunk(1, 3)                          # k2-5
            for b in range(4):
                wchunk(nc.sync, b, 4, 10)        # k4-9 cols, 192KB each
            wchunk(nc.scalar, 4, 4, 10)
            wchunk(nc.scalar, 5, 4, 10)
            xchunk(3, 6)                          # k6-11
            for b in range(4):
                wchunk(nc.sync, b, 10, 16)       # k10-15 cols
            wchunk(nc.scalar, 4, 10, 16)
            wchunk(nc.scalar, 5, 10, 16)
            xchunk(6, 8)                          # k12-15

            # t=1..3 weight groups, time-gated so the scheduler streams
            # them just ahead of consumption; t=1's chunks are finer so its
            # first tiles complete before group t=0 finishes
            wgt = {}
            for t in (1, 2, 3):
                wg = wgpool.tile([128, 6 * W], ACT, tag="wg")
                if t == 1:
                    with tc.tile_wait_until(0.010):
                        for c in range(4):
                            nc.sync.dma_start(
                                wg[:, bass.ts(c, 3 * W // 2)],
                                ew1g[t][:, bass.ts(c, 3 * W // 2)])
                else:
                    with tc.tile_wait_until(0.012 + 0.020 * (t - 1)):
                        nc.sync.dma_start(wg[:, :3 * W], ew1g[t][:, :3 * W])
                        nc.sync.dma_start(wg[:, 3 * W:], ew1g[t][:, 3 * W:])
                wgt[t] = wg

            def xslice(k):
                return xall[:, (k // 2) * 1024 + (k % 2) * ROWS:][:, :ROWS]

            btile = bpool.tile([128, 2 * MT + WT], F32, tag="biases")
            nc.gpsimd.dma_start(btile[:], biases[:])

            def sb1_col(j):
                return btile[:, j:j + 1]

            def sb2_col(j):
                return btile[:, MT + j:MT + j + 1]

            def bo2_col(j):
                return btile[:, 2 * MT + j:2 * MT + j + 1]

            def elementwise(St, first):
                """complex triple product on one partition-row group."""
                qA, qB, kA, kB, vA, vB = St
                v = nc.vector
                cr = ewpool.tile([128, ROWS], F32, tag="ew")
                ci = ewpool.tile([128, ROWS], F32, tag="ew")
                t0 = ewpool.tile([128, ROWS], F32, tag="ew")
                v.tensor_mul(cr[:], kA[:], qA[:])
                v.tensor_mul(t0[:], kB[:], qB[:])
                v.tensor_add(cr[:], cr[:], t0[:])
                v.tensor_mul(ci[:], kB[:], qA[:])
                v.tensor_mul(t0[:], kA[:], qB[:])
                v.tensor_sub(ci[:], ci[:], t0[:])
                wr = wtpool.tile([128, ROWS], ACT, tag="wt")
                wi = wtpool.tile([128, ROWS], ACT, tag="wt")
                v.tensor_mul(wr[:], cr[:], vA[:])
                v.tensor_mul(t0[:], ci[:], vB[:])
                v.tensor_sub(wr[:], wr[:], t0[:])
                v.tensor_mul(wi[:], cr[:], vB[:])
                v.tensor_mul(t0[:], ci[:], vA[:])
                v.tensor_add(wi[:], wi[:], t0[:])
                if first:
                    # slot 0: A holds DC, B holds Nyquist — both real
                    v.tensor_mul(t0[0:1, :], qA[0:1, :], kA[0:1, :])
                    v.tensor_mul(wr[0:1, :], t0[0:1, :], vA[0:1, :])
                    v.tensor_mul(t0[0:1, :], qB[0:1, :], kB[0:1, :])
                    v.tensor_mul(wi[0:1, :], t0[0:1, :], vB[0:1, :])
                return wr, wi

            # ---- stage 1: S1 = x @ EW1 + b1, pipelined elementwise ----
            Wcat1 = [None] * ST
            for t in range(4):
                St = []
                if t == 0:
                    # 6-way-interleaved k-loop while x streams
                    psq = []
                    for _ in range(6):
                        ps = pspool.tile([128, ROWS], F32, tag="ps")
                        psq.append(ps)
                    for k in range(KT):
                        for i in range(6):
                            nc.tensor.matmul(
                                psq[i][:], wq0[:, i * W + k * 128:][:, :128],
                                xslice(k), start=(k == 0), stop=(k == KT - 1))
                    for i in range(6):
                        Sj = spool.tile([128, ROWS], F32, tag="spec")
                        nc.scalar.activation(Sj[:], psq[i][:], IDENT,
                                             bias=sb1_col(4 * i))
                        St.append(Sj)
                else:
                    for b in range(6):   # qA qB kA kB vA vB row t
                        j = b * 4 + t
                        ps = pspool.tile([128, ROWS], F32, tag="ps")
                        for k in range(KT):
                            nc.tensor.matmul(
                                ps[:],
                                wgt[t][:, b * W + k * 128:][:, :128],
                                xslice(k), start=(k == 0), stop=(k == KT - 1))
                        Sj = spool.tile([128, ROWS], F32, tag="spec")
                        nc.scalar.activation(Sj[:], ps[:], IDENT,
                                             bias=sb1_col(j))
                        St.append(Sj)
                wr, wi = elementwise(St, t == 0)
                Wcat1[2 * t] = wr
                Wcat1[2 * t + 1] = wi

            # stage-2/3 weights prefetch as few large transfers, time-gated
            # (tile_wait_until holds them back in the Tile scheduler) so
            # they can't jump ahead of the stage-1 weight stream and steal
            # HBM bandwidth from the critical path
            w2t = []
            for t in range(4):
                w2 = wmpool.tile([128, 6 * P], ACT, tag="wm")
                with tc.tile_wait_until(0.048 + 0.008 * t):
                    nc.sync.dma_start(w2[:], m12g[t])
                w2t.append(w2)
            wo2t = []
            for c in range(2):
                wo2 = wo2pool.tile([128, 8 * P], ACT, tag="wo2")
                with tc.tile_wait_until(0.085 + 0.010 * c):
                    nc.scalar.dma_start(wo2[:], wo2g[c])
                wo2t.append(wo2)

            # ---- stage 2: S2 = Wt1 @ M12 + b2, pipelined elementwise ----
            Wcat2 = [None] * ST
            for t in range(4):
                St = []
                for b in range(6):
                    j = b * 4 + t
                    ps = pspool.tile([128, ROWS], F32, tag="ps")
                    for s in range(ST):
                        nc.tensor.matmul(
                            ps[:], w2t[t][:, b * P + s * 128:][:, :128],
                            Wcat1[s][:], start=(s == 0), stop=(s == ST - 1))
                    Sj = spool.tile([128, ROWS], F32, tag="spec")
                    nc.scalar.activation(Sj[:], ps[:], IDENT,
                                         bias=sb2_col(j))
                    St.append(Sj)
                wr, wi = elementwise(St, t == 0)
                Wcat2[2 * t] = wr
                Wcat2[2 * t + 1] = wi

            # ---- stage 3: out = Wt2 @ WoP2 + bo2 ----
            # output tiles drain in PAIRS into one SBUF tile (one DMA per
            # pair halves the out-DMA count and the teardown walk);
            # alternate drain engines so consecutive tiles' bias-add +
            # copy pipeline instead of serializing on one engine
            for jp in range(WT // 2):
                last_pair = jp == WT // 2 - 1
                ostage = opool.tile([128, 2 * ROWS], ACT, tag="out")
                for i in range(2):
                    j = 2 * jp + i
                    ps = pspool.tile([128, ROWS], F32, tag="ps")
                    for s in range(ST):
                        nc.tensor.matmul(
                            ps[:],
                            wo2t[j // 8][:, (j % 8) * P + s * 128:][:, :128],
                            Wcat2[s][:], start=(s == 0), stop=(s == ST - 1))
                    half = ostage[:, i * ROWS:][:, :ROWS]
                    if last_pair and i == 1:
                        # the very last tile's drain chain IS the exec-time
                        # endpoint: split it into two 256-row halves so the
                        # first half's (small) DMA overlaps the second
                        # half's bias-add, on both HWDGE queues
                        hs = ROWS // 2
                        for hh in range(2):
                            nc.vector.tensor_scalar_add(
                                half[:, hh * hs:][:, :hs],
                                ps[:, hh * hs:][:, :hs], bo2_col(j))
                            (nc.sync if hh == 0 else nc.scalar).dma_start(
                                xout[jp][:, i * ROWS + hh * hs:][:, :hs],
                                half[:, hh * hs:][:, :hs])
                        continue
                    if i:
                        nc.vector.tensor_scalar_add(half, ps[:], bo2_col(j))
                    else:
                        nc.scalar.activation(half, ps[:], IDENT,
                                             bias=bo2_col(j))
                    if last_pair:
                        (nc.sync if i == 0 else nc.scalar).dma_start(
                            xout[jp][:, i * ROWS:][:, :ROWS], half)
                if not last_pair:
                    (nc.scalar if jp % 2 else nc.sync).dma_start(
                        xout[jp], ostage[:])
    nc.compile()
    return nc


_MODULE_CACHE = {}


def _get_module():
    if "nc" not in _MODULE_CACHE:
        _MODULE_CACHE["nc"] = _build_module()
    return _MODULE_CACHE["nc"]


def _prepare_weight_maps(conv_w, conv_b, wq, bq, wk, bk, wv, bv, wo, bo):
    folds = [_fold_layer(conv_w[l], conv_b[l], wq[l], bq[l], wk[l], bk[l],
                         wv[l], bv[l], wo[l], bo[l]) for l in range(L)]
    EW1, Sb1, WoP1, _bo1 = folds[0]
    EW2, Sb2, WoP2, bo2 = folds[1]
    M12 = WoP1 @ EW2                               # [P, 3P], fp64
    Sb2e = _bo1 @ EW2 + Sb2                        # [3P]

    # ew1 grouped by elementwise group t (j = b*4 + t, b-major):
    # ew1g[t][p, b*2048 + k*128 + c] = EW1[k*128+p, (b*4+t)*128+c]
    arr1 = EW1.reshape(KT, 128, MT // 4, 4, 128)
    ew1g = np.ascontiguousarray(
        arr1.transpose(3, 1, 2, 0, 4).reshape(4, 128, 6 * W).astype(ACT_NP))

    # m12 grouped by elementwise group t (j = b*4 + t, b-major):
    # m12g[t][p, b*1024 + s*128 + c] = M12[s*128+p, (b*4+t)*128+c]
    arr2 = M12.reshape(ST, 128, MT // 4, 4, 128)
    m12g = np.ascontiguousarray(
        arr2.transpose(3, 1, 2, 0, 4).reshape(4, 128, 6 * P).astype(ACT_NP))

    # wop2 in 2 chunks of 8 out-tiles:
    # wo2g[c][p, jj*1024 + s*128 + cc] = WoP2[s*128+p, (8c+jj)*128+cc]
    arr3 = WoP2.reshape(ST, 128, 2, 8, 128)
    wo2g = np.ascontiguousarray(
        arr3.transpose(2, 1, 3, 0, 4).reshape(2, 128, 8 * P).astype(ACT_NP))

    biases = np.concatenate([
        Sb1.reshape(MT, 128).T, Sb2e.reshape(MT, 128).T,
        bo2.reshape(WT, 128).T], axis=1).astype(np.float32)
    return {
        "ew1g": ew1g,
        "m12g": m12g,
        "wo2g": wo2g,
        "biases": np.ascontiguousarray(biases),
    }


def _make_in_maps(inputs):
    x = np.asarray(inputs["x"], dtype=np.float32)
    wmap = _prepare_weight_maps(
        np.asarray(inputs["conv_w"]), np.asarray(inputs["conv_b"]),
        np.asarray(inputs["wq"]), np.asarray(inputs["bq"]),
        np.asarray(inputs["wk"]), np.asarray(inputs["bk"]),
        np.asarray(inputs["wv"]), np.asarray(inputs["bv"]),
        np.asarray(inputs["wo"]), np.asarray(inputs["bo"]))
    per_core = BS // NCORES
    in_maps = []
    for c in range(NCORES):
        xc = x[c * per_core:(c + 1) * per_core].reshape(ROWS, W)
        xin = np.ascontiguousarray(
            xc.reshape(ROWS, KT // 2, 2, 128).transpose(3, 1, 2, 0)
            .reshape(128, KT * ROWS).astype(ACT_NP))
        in_maps.append({"xin": xin, **wmap})
    return in_maps


def kernel(x, conv_w, conv_b, wq, bq, wk, bk, wv, bv, wo, bo):
    in_maps = _make_in_maps(dict(
        x=x, conv_w=conv_w, conv_b=conv_b, wq=wq, bq=bq, wk=wk, bk=bk,
        wv=wv, bv=bv, wo=wo, bo=bo))
    nc = _get_module()
    res = run_bass_kernel_spmd(nc, in_maps, list(range(NCORES)))

    per_core = BS // NCORES
    outs = []
    for c in range(NCORES):
        xo = res.results[c]["xout"]                    # [WT//2, 128, 2*ROWS]
        xo = (xo.reshape(WT // 2, 128, 2, ROWS).transpose(0, 2, 1, 3)
              .reshape(WT, 128, ROWS))
        outs.append(xo.transpose(2, 0, 1).reshape(per_core, CNT, W))
    return np.concatenate(outs, axis=0).astype(np.float32)



# revision 54
# speedup vs baseline: 1.0020x; 1.0019x over previous
"""Trainium2 Bass kernel for nn_FEDformerEncoder (8-core data parallel).

The reference network is, per layer (L=2):
    y  = mean_e( conv1d_same(x, w_e) + b_e )              (depthwise conv on W)
    q,k,v = y @ w{q,k,v}.T + b{q,k,v}                     ([rows, P])
    Q,K,V = fft(q),fft(k),fft(v)
    Wt = K * conj(Q) / sqrt(P) * V
    out = ifft(Wt).real @ wo.T + bo

Everything except the elementwise complex triple product is linear in x, so
the conv, the FFT, and the iFFT fold into host-precomputed projection
weights.  Real-input FFT symmetry packs each 1024-bin complex spectrum into
exactly 1024 reals per signal: block A = Re[0..511], block B =
[Re[512](Nyquist), Im[1..511]].  Composing the two layers' linear maps
(iFFT-projection of layer 1 directly into conv+FFT-projection of layer 2)
collapses the whole network into three matmul stages and two elementwise
stages:

    S1  = x   @ EW1  + b1     # [rows,2048] @ [2048,3072]
    Wt1 = complex-triple(S1)  # packed; slot 0 of A/B = DC/Nyquist, real
    S2  = Wt1 @ M12  + b2     # [rows,1024] @ [1024,3072], M12 = WoP1@EW2
    Wt2 = complex-triple(S2)
    out = Wt2 @ WoP2 + bo2    # [rows,1024] @ [1024,2048]

Sharded batch-wise over 8 cores (4 batches = 512 rows per core), weights
replicated.  Activations live in SBUF in transposed layout [feature(part),
row(free)] throughout, so no on-device transposes are needed.  Matmul
operands are fp16 (fp32 PSUM accumulation, fp32 elementwise); contraction
row-tiles are interleaved [A0 B0 A1 B1 ...] so each elementwise group
feeds the next stage in production order and the stages pipeline.

The kernel is PE-bound: 704 matmul tiles x 512 fp16 rows = 150.2us of
tensor-engine work at 2.4GHz, so everything else is scheduled around a
gapless PE run:

- HAM warm-up: dummy matmuls (no DMA deps) start right after the ~7.2us
  runtime prologue so the PE's duty-cycle clock gate (cold K=4/8)
  releases around the time the first real data lands.
- Group t=0 of stage 1 runs all six output tiles as ONE 6-way
  interleaved k-loop: per 128KB x k-slice the PE does 6 matmuls, so x
  HBM demand is ~100GB/s and weights ~150GB/s — comfortably under the
  ~358GB/s HBM-per-core limit while x streams in.
- Weights are pre-grouped in DRAM by elementwise group (host repack)
  and stream in k-need order: small first chunks (PE start ~11.5us),
  then large transfers.  Later groups' loads are time-gated
  (tile_wait_until) so the Tile scheduler cannot hoist them ahead of
  the critical stream and oversubscribe HBM — sustained >300GB/s DMA
  also risks the chip's P0 power downclock (PE 2.4 -> 2.0GHz).
- Output is written fp16 (cast to fp32 on host); rel err stays ~1.2e-3.
"""
import sys

import numpy as np

sys.path.insert(0, "/opt/trn_rl_repo")

import concourse.bass as bass
import concourse.mybir as mybir
import concourse.tile as tile
from concourse import bacc
from concourse.bass_utils import run_bass_kernel_spmd

BS, CNT, W, P, E, KK, L = 32, 128, 2048, 1024, 8, 25, 2
H = P // 2                    # 512 slots per packed block
NCORES = 8
ROWS = (BS // NCORES) * CNT   # 512 rows per core
KT = W // 128                 # 16 contraction tiles (stage 1)
MT = (3 * P) // 128           # 24 output tiles (stages 1,2: q|k|v packed)
ST = P // 128                 # 8 contraction tiles (stages 2,3)
WT = W // 128                 # 16 output tiles (stage 3)
F32 = mybir.dt.float32
ACT = mybir.dt.float16
ACT_NP = np.float16
IDENT = mybir.ActivationFunctionType.Identity


def _fold_layer(conv_w, conv_b, wq, bq, wk, bk, wv, bv, wo, bo):
    """Fold conv + FFT into projection weights (float64 math).

    Returns EW [W, 3*P], Sbias [3*P], WoP [P, W] (rows interleaved
    [A0 B0 A1 B1 A2 B2 A3 B3] by 128-tile), bo [W].
    """
    f64 = np.float64
    wbar = conv_w.astype(f64).mean(axis=0)[0]          # [KK]
    bbar = conv_b.astype(f64).mean()

    idx = np.arange(W)
    D = idx[None, :] - idx[:, None] + (KK // 2)        # C[w,u] = wbar[u-w+12]
    C = np.where((D >= 0) & (D < KK), wbar[np.clip(D, 0, KK - 1)], 0.0)

    def pack_fwd(wmat, bvec, scale=1.0):
        Wf = np.fft.fft(wmat.astype(f64), axis=0)      # [P, W]
        Bf = np.fft.fft(bvec.astype(f64))              # [P]
        cols = np.empty((W, P), dtype=f64)
        cols[:, :H] = Wf[:H, :].real.T
        cols[:, H] = Wf[H, :].real
        cols[:, H + 1:] = Wf[1:H, :].imag.T
        bias = np.empty(P, dtype=f64)
        bias[:H] = Bf[:H].real
        bias[H] = Bf[H].real
        bias[H + 1:] = Bf[1:H].imag
        return cols * scale, bias * scale

    s = 1.0 / np.sqrt(f64(P))
    cq, bq_p = pack_fwd(wq, bq)
    ck, bk_p = pack_fwd(wk, bk)
    cv, bv_p = pack_fwd(wv, bv, scale=s)
    cols = np.concatenate([cq, ck, cv], axis=1)        # [W, 3P]
    bias = np.concatenate([bq_p, bk_p, bv_p])

    EW = C.T @ cols
    Sbias = bbar * cols.sum(axis=0) + bias

    G = np.fft.ifft(wo.astype(f64), axis=1)            # [W, P]
    WoP = np.empty((P, W), dtype=f64)
    WoP[0] = G[:, 0].real
    WoP[1:H] = 2.0 * G[:, 1:H].real.T
    WoP[H] = G[:, H].real
    WoP[H + 1:] = -2.0 * G[:, 1:H].imag.T
    # interleave row-tiles A0 B0 A1 B1 ... to match Wcat production order
    WoP = WoP.reshape(2, 4, 128, W).transpose(1, 0, 2, 3).reshape(P, W)

    return EW, Sbias, WoP, bo.astype(f64)


def _build_module():
    nc = bacc.Bacc("TRN2", target_bir_lowering=False, debug=False)

    # x, k-pair-major: col = kp*1024 + half*512 + row  (k = 2*kp + half)
    xin = nc.dram_tensor("xin", [128, KT * ROWS], ACT, kind="ExternalInput")
    # stage-1 weights grouped by elementwise group t: col = b*2048 + k*128
    ew1g = nc.dram_tensor("ew1g", [4, 128, 6 * W], ACT, kind="ExternalInput")
    # stage-2 weights grouped by elementwise group t2: col = b*1024 + s*128
    m12g = nc.dram_tensor("m12g", [4, 128, 6 * P], ACT, kind="ExternalInput")
    # stage-3 weights in 2 chunks of 8 out-tiles: col = jj*1024 + s*128
    wo2g = nc.dram_tensor("wo2g", [2, 128, 8 * P], ACT, kind="ExternalInput")
    # [sb1(24) | sb2(24) | bo2(16)] packed into one [128, 64] tensor
    biases = nc.dram_tensor("biases", [128, 2 * MT + WT], F32,
                            kind="ExternalInput")
    # output pair-major: xout[jp][:, i*ROWS + r] = out tile j=2*jp+i
    xout = nc.dram_tensor("xout", [WT // 2, 128, 2 * ROWS], ACT,
                          kind="ExternalOutput")

    with tile.TileContext(nc) as tc:
        with (
            tc.tile_pool(name="xbuf", bufs=1) as xpool,
            tc.tile_pool(name="bias", bufs=1) as bpool,
            tc.tile_pool(name="wq0", bufs=1) as wqpool,
            tc.tile_pool(name="wg", bufs=2) as wgpool,
            tc.tile_pool(name="wm", bufs=2) as wmpool,
            tc.tile_pool(name="wo2", bufs=2) as wo2pool,
            tc.tile_pool(name="spec", bufs=10) as spool,
            tc.tile_pool(name="wt", bufs=16) as wtpool,
            tc.tile_pool(name="ew", bufs=6) as ewpool,
            tc.tile_pool(name="out", bufs=4) as opool,
            tc.tile_pool(name="psum", bufs=8, space="PSUM") as pspool,
        ):
            # first stage-1 weight blocks go out before x so the PE can
            # start as soon as x k-tiles stream in; x loads issue on the
            # (idle-at-start) scalar engine's DGE, weights on sync's.
            # The two t=0 tiles' chunks are issue-interleaved so BOTH have
            # their first half-tile early: the t=0 k-loop alternates j=0/j=4
            # matmuls, so it stalls on whichever tile lands later.  Each
            # DMA instruction costs ~0.65us of issue time on its engine, so
            # chunks are as few and as large as latency allows.
            # PE warm-up: the HAM clock gate holds a cold tensor engine at
            # half duty (K=4/8) until it sees several us of sustained
            # activity.  These matmuls depend on no DMA, so they run right
            # after the runtime prologue and release the gate around the
            # time real data lands.
            scratch = bpool.tile([128, ROWS], ACT, tag="warm")
            nc.gpsimd.memset(scratch[:], 0.0)
            wps = pspool.tile([128, ROWS], F32, tag="ps")
            for _ in range(13):
                nc.tensor.matmul(wps[:], scratch[:, 0:128], scratch[:],
                                 start=True, stop=True)

            # t=0 runs ALL SIX tiles as one 6-way-interleaved k-loop: per k
            # step that's 6 matmuls per 128KB x slice, so x HBM demand is
            # ~100GB/s and weights ~150GB/s — comfortably under the
            # ~358GB/s HBM-per-core limit even at full PE clock, leaving
            # slack for DMA jitter.  Chunks are small and k-need-ordered
            # (all six tiles' first 512 cols, then k-range halves) so the
            # k-loop starts early and never outruns the stream.  b=0..3 on
            # sync, b=4..5 + x on scalar.
            wq0 = wqpool.tile([128, 6 * W], ACT, tag="wq")
            xall = xpool.tile([128, KT * ROWS], ACT, tag="x")
            qx = ROWS * 2

            def wchunk(eng, b, c0, c1):
                eng.dma_start(wq0[:, b * W + c0 * 128:][:, :(c1 - c0) * 128],
                              ew1g[0][:, b * W + c0 * 128:][:, :(c1 - c0) * 128])

            def xchunk(c0, c1):
                nc.scalar.dma_start(xall[:, c0 * qx:c1 * qx],
                                    xin[:, c0 * qx:c1 * qx])

            for b in range(4):
                wchunk(nc.sync, b, 0, 4)         # k0-3 cols, 128KB each
            xchunk(0, 1)                          # k0,k1
            wchunk(nc.scalar, 4, 0, 4)
            wchunk(nc.scalar, 5, 0, 4)
            xchunk(1, 3)                          # k2-5
            for b in range(4):
                wchunk(nc.sync, b, 4, 10)        # k4-9 cols, 192KB each
            wchunk(nc.scalar, 4, 4, 10)
            wchunk(nc.scalar, 5, 4, 10)
            xchunk(3, 6)                          # k6-11
            for b in range(4):
                wchunk(nc.sync, b, 10, 16)       # k10-15 cols
            wchunk(nc.scalar, 4, 10, 16)
            wchunk(nc.scalar, 5, 10, 16)
            xchunk(6, 8)                          # k12-15

            # t=1..3 weight groups, time-gated so the scheduler streams
            # them just ahead of consumption; t=1's chunks are finer so its
            # first tiles complete before group t=0 finishes
            wgt = {}
            for t in (1, 2, 3):
                wg = wgpool.tile([128, 6 * W], ACT, tag="wg")
                if t == 1:
                    with tc.tile_wait_until(0.010):
                        for c in range(4):
                            nc.sync.dma_start(
                                wg[:, bass.ts(c, 3 * W // 2)],
                                ew1g[t][:, bass.ts(c, 3 * W // 2)])
                else:
                    with tc.tile_wait_until(0.012 + 0.020 * (t - 1)):
                        nc.sync.dma_start(wg[:, :3 * W], ew1g[t][:, :3 * W])
                        nc.sync.dma_start(wg[:, 3 * W:], ew1g[t][:, 3 * W:])
                wgt[t] = wg

            def xslice(k):
                return xall[:, (k // 2) * 1024 + (k % 2) * ROWS:][:, :ROWS]

            btile = bpool.tile([128, 2 * MT + WT], F32, tag="biases")
            nc.gpsimd.dma_start(btile[:], biases[:])

            def sb1_col(j):
                return btile[:, j:j + 1]

            def sb2_col(j):
                return btile[:, MT + j:MT + j + 1]

            def bo2_col(j):
                return btile[:, 2 * MT + j:2 * MT + j + 1]

            def elementwise(St, first):
                """complex triple product on one partition-row group."""
                qA, qB, kA, kB, vA, vB = St
                v = nc.vector
                cr = ewpool.tile([128, ROWS], F32, tag="ew")
                ci = ewpool.tile([128, ROWS], F32, tag="ew")
                t0 = ewpool.tile([128, ROWS], F32, tag="ew")
                v.tensor_mul(cr[:], kA[:], qA[:])
                v.tensor_mul(t0[:], kB[:], qB[:])
                v.tensor_add(cr[:], cr[:], t0[:])
                v.tensor_mul(ci[:], kB[:], qA[:])
                v.tensor_mul(t0[:], kA[:], qB[:])
                v.tensor_sub(ci[:], ci[:], t0[:])
                wr = wtpool.tile([128, ROWS], ACT, tag="wt")
                wi = wtpool.tile([128, ROWS], ACT, tag="wt")
                v.tensor_mul(wr[:], cr[:], vA[:])
                v.tensor_mul(t0[:], ci[:], vB[:])
                v.tensor_sub(wr[:], wr[:], t0[:])
                v.tensor_mul(wi[:], cr[:], vB[:])
                v.tensor_mul(t0[:], ci[:], vA[:])
                v.tensor_add(wi[:], wi[:], t0[:])
                if first:
                    # slot 0: A holds DC, B holds Nyquist — both real
                    v.tensor_mul(t0[0:1, :], qA[0:1, :], kA[0:1, :])
                    v.tensor_mul(wr[0:1, :], t0[0:1, :], vA[0:1, :])
                    v.tensor_mul(t0[0:1, :], qB[0:1, :], kB[0:1, :])
                    v.tensor_mul(wi[0:1, :], t0[0:1, :], vB[0:1, :])
                return wr, wi

            # ---- stage 1: S1 = x @ EW1 + b1, pipelined elementwise ----
            Wcat1 = [None] * ST
            for t in range(4):
                St = []
                if t == 0:
                    # 6-way-interleaved k-loop while x streams
                    psq = []
                    for _ in range(6):
                        ps = pspool.tile([128, ROWS], F32, tag="ps")
                        psq.append(ps)
                    for k in range(KT):
                        for i in range(6):
                            nc.tensor.matmul(
                                psq[i][:], wq0[:, i * W + k * 128:][:, :128],
                                xslice(k), start=(k == 0), stop=(k == KT - 1))
                    for i in range(6):
                        Sj = spool.tile([128, ROWS], F32, tag="spec")
                        nc.scalar.activation(Sj[:], psq[i][:], IDENT,
                                             bias=sb1_col(4 * i))
                        St.append(Sj)
                else:
                    for b in range(6):   # qA qB kA kB vA vB row t
                        j = b * 4 + t
                        ps = pspool.tile([128, ROWS], F32, tag="ps")
                        for k in range(KT):
                            nc.tensor.matmul(
                                ps[:],
                                wgt[t][:, b * W + k * 128:][:, :128],
                                xslice(k), start=(k == 0), stop=(k == KT - 1))
                        Sj = spool.tile([128, ROWS], F32, tag="spec")
                        nc.scalar.activation(Sj[:], ps[:], IDENT,
                                             bias=sb1_col(j))
                        St.append(Sj)
                wr, wi = elementwise(St, t == 0)
                Wcat1[2 * t] = wr
                Wcat1[2 * t + 1] = wi

            # stage-2/3 weights prefetch as few large transfers, time-gated
            # (tile_wait_until holds them back in the Tile scheduler) so
            # they can't jump ahead of the stage-1 weight stream and steal
            # HBM bandwidth from the critical path
            w2t = []
            for t in range(4):
                w2 = wmpool.tile([128, 6 * P], ACT, tag="wm")
                with tc.tile_wait_until(0.048 + 0.008 * t):
                    nc.sync.dma_start(w2[:], m12g[t])
                w2t.append(w2)
            wo2t = []
            for c in range(2):
                wo2 = wo2pool.tile([128, 8 * P], ACT, tag="wo2")
                with tc.tile_wait_until(0.085 + 0.010 * c):
                    nc.scalar.dma_start(wo2[:], wo2g[c])
                wo2t.append(wo2)

            # ---- stage 2: S2 = Wt1 @ M12 + b2, pipelined elementwise ----
            Wcat2 = [None] * ST
            for t in range(4):
                St = []
                for b in range(6):
                    j = b * 4 + t
                    ps = pspool.tile([128, ROWS], F32, tag="ps")
                    for s in range(ST):
                        nc.tensor.matmul(
                            ps[:], w2t[t][:, b * P + s * 128:][:, :128],
                            Wcat1[s][:], start=(s == 0), stop=(s == ST - 1))
                    Sj = spool.tile([128, ROWS], F32, tag="spec")
                    nc.scalar.activation(Sj[:], ps[:], IDENT,
                                         bias=sb2_col(j))
                    St.append(Sj)
                wr, wi = elementwise(St, t == 0)
                Wcat2[2 * t] = wr
                Wcat2[2 * t + 1] = wi

            # ---- stage 3: out = Wt2 @ WoP2 + bo2 ----
            # output tiles drain in PAIRS into one SBUF tile (one DMA per
            # pair halves the out-DMA count and the teardown walk);
            # alternate drain engines so consecutive tiles' bias-add +
            # copy pipeline instead of serializing on one engine
            for jp in range(WT // 2):
                last_pair = jp == WT // 2 - 1
                ostage = opool.tile([128, 2 * ROWS], ACT, tag="out")
                for i in range(2):
                    j = 2 * jp + i
                    ps = pspool.tile([128, ROWS], F32, tag="ps")
                    for s in range(ST):
                        nc.tensor.matmul(
                            ps[:],
                            wo2t[j // 8][:, (j % 8) * P + s * 128:][:, :128],
                            Wcat2[s][:], start=(s == 0), stop=(s == ST - 1))
                    half = ostage[:, i * ROWS:][:, :ROWS]
                    if i:
                        nc.vector.tensor_scalar_add(half, ps[:], bo2_col(j))
                    else:
                        nc.scalar.activation(half, ps[:], IDENT,
                                             bias=bo2_col(j))
                    if last_pair:
                        # final tiles ship as two half-DMAs on BOTH queues
                        # so the drain chains overlap and the last transfer
                        # is small — this chain is the exec-time endpoint
                        (nc.sync if i == 0 else nc.scalar).dma_start(
                            xout[jp][:, i * ROWS:][:, :ROWS], half)
                if not last_pair:
                    (nc.scalar if jp % 2 else nc.sync).dma_start(
                        xout[jp], ostage[:])
    nc.compile()
    return nc


_MODULE_CACHE = {}


def _get_module():
    if "nc" not in _MODULE_CACHE:
        _MODULE_CACHE["nc"] = _build_module()
    return _MODULE_CACHE["nc"]


def _prepare_weight_maps(conv_w, conv_b, wq, bq, wk, bk, wv, bv, wo, bo):
    folds = [_fold_layer(conv_w[l], conv_b[l], wq[l], bq[l], wk[l], bk[l],
                         wv[l], bv[l], wo[l], bo[l]) for l in range(L)]
    EW1, Sb1, WoP1, _bo1 = folds[0]
    EW2, Sb2, WoP2, bo2 = folds[1]
    M12 = WoP1 @ EW2                               # [P, 3P], fp64
    Sb2e = _bo1 @ EW2 + Sb2                        # [3P]

    # ew1 grouped by elementwise group t (j = b*4 + t, b-major):
    # ew1g[t][p, b*2048 + k*128 + c] = EW1[k*128+p, (b*4+t)*128+c]
    arr1 = EW1.reshape(KT, 128, MT // 4, 4, 128)
    ew1g = np.ascontiguousarray(
        arr1.transpose(3, 1, 2, 0, 4).reshape(4, 128, 6 * W).astype(ACT_NP))

    # m12 grouped by elementwise group t (j = b*4 + t, b-major):
    # m12g[t][p, b*1024 + s*128 + c] = M12[s*128+p, (b*4+t)*128+c]
    arr2 = M12.reshape(ST, 128, MT // 4, 4, 128)
    m12g = np.ascontiguousarray(
        arr2.transpose(3, 1, 2, 0, 4).reshape(4, 128, 6 * P).astype(ACT_NP))

    # wop2 in 2 chunks of 8 out-tiles:
    # wo2g[c][p, jj*1024 + s*128 + cc] = WoP2[s*128+p, (8c+jj)*128+cc]
    arr3 = WoP2.reshape(ST, 128, 2, 8, 128)
    wo2g = np.ascontiguousarray(
        arr3.transpose(2, 1, 3, 0, 4).reshape(2, 128, 8 * P).astype(ACT_NP))

    biases = np.concatenate([
        Sb1.reshape(MT, 128).T, Sb2e.reshape(MT, 128).T,
        bo2.reshape(WT, 128).T], axis=1).astype(np.float32)
    return {
        "ew1g": ew1g,
        "m12g": m12g,
        "wo2g": wo2g,
        "biases": np.ascontiguousarray(biases),
    }


def _make_in_maps(inputs):
    x = np.asarray(inputs["x"], dtype=np.float32)
    wmap = _prepare_weight_maps(
        np.asarray(inputs["conv_w"]), np.asarray(inputs["conv_b"]),
        np.asarray(inputs["wq"]), np.asarray(inputs["bq"]),
        np.asarray(inputs["wk"]), np.asarray(inputs["bk"]),
        np.asarray(inputs["wv"]), np.asarray(inputs["bv"]),
        np.asarray(inputs["wo"]), np.asarray(inputs["bo"]))
    per_core = BS // NCORES
    in_maps = []
    for c in range(NCORES):
        xc = x[c * per_core:(c + 1) * per_core].reshape(ROWS, W)
        xin = np.ascontiguousarray(
            xc.reshape(ROWS, KT // 2, 2, 128).transpose(3, 1, 2, 0)
            .reshape(128, KT * ROWS).astype(ACT_NP))
        in_maps.append({"xin": xin, **wmap})
    return in_maps


def kernel(x, conv_w, conv_b, wq, bq, wk, bk, wv, bv, wo, bo):
    in_maps = _make_in_maps(dict(
        x=x, conv_w=conv_w, conv_b=conv_b, wq=wq, bq=bq, wk=wk, bk=bk,
        wv=wv, bv=bv, wo=wo, bo=bo))
    nc = _get_module()
    res = run_bass_kernel_spmd(nc, in_maps, list(range(NCORES)))

    per_core = BS // NCORES
    outs = []
    for c in range(NCORES):
        xo = res.results[c]["xout"]                    # [WT//2, 128, 2*ROWS]
        xo = (xo.reshape(WT // 2, 128, 2, ROWS).transpose(0, 2, 1, 3)
              .reshape(WT, 128, ROWS))
        outs.append(xo.transpose(2, 0, 1).reshape(per_core, CNT, W))
    return np.concatenate(outs, axis=0).astype(np.float32)

